# revision 1
# baseline (speedup 1.0000x reference)
"""Trainium2 Bass kernel: per-edge dot product (u_dot_v GNN predictor).

score[e] = sum_d h[src[e], d] * h[dst[e], d]   -> [E, 1] float32

Strategy (edge-parallel over 8 cores):
  - Each core gets E/8 = 80000 edges; the full node table h stays in HBM and
    rows are fetched per edge with the GPSIMD dma_gather instruction.
    The kernel is bound by the Q7 descriptor-generation rate (~8 ns/descriptor),
    so the design minimizes descriptor count.
  - dma_gather indices are int16, so node ids >= 32768 can't be addressed
    directly. Host buckets each core's edges 4 ways by (src >= 32768,
    dst >= 32768); each bucket's gathers use base-offset views of h with
    rebased indices.
  - Descriptor reduction: within each bucket, edges whose src rows are
    CONSECUTIVE (r, r+1) are paired; one elem_size=256/elem_step=128
    descriptor fetches both rows (overlapping-window source AP), so a pair
    of edges costs 1 src descriptor instead of 2. ~70% of edges pair up.
  - Per tile of 1024 descriptors: gather h[src] (paired or single) and the
    matching h[dst] rows, DVE multiply + segmented reduce over the feature
    axis -> [128, 8] scores per 1024 edge-slots.
  - Host un-permutes the bucketed scores back to edge order.
"""

import numpy as np

import concourse.bacc as bacc
import concourse.mybir as mybir
import concourse.tile as tile
from concourse import bass
from concourse.bass_utils import run_bass_kernel_spmd

N_NODES = 50000
D = 128
N_EDGES = 640000
N_CORES = 8
P = 128
E_CORE = N_EDGES // N_CORES  # 80000
NI = 1024                    # descriptors per dma_gather tile
C = NI // P                  # 8 chunks per partition
S = NI // 16                 # idx columns per 16-partition block
OFF = 32768                  # int16 index range boundary

_CACHE: dict = {}


def _h_tables(nc, h):
    """Per-bucket source APs: (single-row table, overlapping pair table)."""
    lo_rows, hi_rows = OFF, N_NODES - OFF
    h_lo1 = h[:OFF, :]
    h_hi1 = h[OFF:, :]
    # overlapping-window pair tables: pseudo-row r = elements 128r..128r+255
    # (rows r and r+1). Base stride 128 elems, window 256 elems.
    h_lo2 = bass.AP(h.tensor, 0, [[D, lo_rows - 1], [1, 2 * D]])
    h_hi2 = bass.AP(h.tensor, OFF * D, [[D, hi_rows - 1], [1, 2 * D]])
    return (h_lo1, h_hi1, h_lo2, h_hi2)


def _build(cfg):
    """cfg = (t2 per group [4], t1 per group [4]) tile counts."""
    t2g, t1g = cfg
    T2, T1 = sum(t2g), sum(t1g)
    nc = bacc.Bacc(
        "TRN2",
        target_bir_lowering=False,
        debug=False,
        enable_asserts=False,
        num_devices=N_CORES,
    )
    h = nc.dram_tensor("h", [N_NODES, D], mybir.dt.float32, kind="ExternalInput").ap()

    def idx_in(name, ntiles):
        return nc.dram_tensor(name, [P, max(ntiles, 1) * S], mybir.dt.int16,
                              kind="ExternalInput").ap()

    sp = idx_in("sp", T2)    # pair src bases
    da = idx_in("da", T2)    # pair dst idx, first edge of pair
    db = idx_in("db", T2)    # pair dst idx, second edge
    ss = idx_in("ss", T1)    # single src idx
    ds = idx_in("ds", T1)    # single dst idx
    ncols = (2 * T2 + T1) * C
    out = nc.dram_tensor("out", [P, ncols], mybir.dt.float32, kind="ExternalOutput").ap()

    h_lo1, h_hi1, h_lo2, h_hi2 = _h_tables(nc, h)
    tab1 = [h_lo1, h_lo1, h_hi1, h_hi1]   # src single table per group
    tab2 = [h_lo2, h_lo2, h_hi2, h_hi2]   # src pair table per group
    dtab = [h_lo1, h_hi1, h_lo1, h_hi1]   # dst table per group

    with tile.TileContext(nc) as tc:
        with (
            tc.tile_pool(name="idx", bufs=1) as ipool,
            tc.tile_pool(name="gath", bufs=4) as gpool,
            tc.tile_pool(name="res", bufs=1) as rpool,
        ):
            def load_idx(ap_dram, ntiles, tag):
                t = ipool.tile([P, max(ntiles, 1) * S], mybir.dt.int16, tag=tag)
                nc.sync.dma_start(out=t[:], in_=ap_dram)
                return t

            sp_sb = load_idx(sp, T2, "sp")
            da_sb = load_idx(da, T2, "da")
            db_sb = load_idx(db, T2, "db")
            ss_sb = load_idx(ss, T1, "ss")
            ds_sb = load_idx(ds, T1, "ds")
            out_sb = rpool.tile([P, ncols], mybir.dt.float32)

            # pair tiles
            t2 = 0
            for g in range(4):
                for _ in range(t2g[g]):
                    isl = slice(t2 * S, (t2 + 1) * S)
                    hu = gpool.tile([P, 2 * NI], mybir.dt.float32, tag="hu2")
                    hva = gpool.tile([P, NI], mybir.dt.float32, tag="hva")
                    hvb = gpool.tile([P, NI], mybir.dt.float32, tag="hvb")
                    nc.gpsimd.dma_gather(
                        out_ap=hu[:].rearrange("p (c d) -> p c d", d=2 * D),
                        in_ap=tab2[g], idxs_ap=sp_sb[:, isl],
                        num_idxs=NI, num_idxs_reg=NI,
                        elem_size=2 * D, elem_step=D,
                    )
                    nc.gpsimd.dma_gather(
                        out_ap=hva[:].rearrange("p (c d) -> p c d", d=D),
                        in_ap=dtab[g], idxs_ap=da_sb[:, isl],
                        num_idxs=NI, num_idxs_reg=NI, elem_size=D,
                    )
                    nc.gpsimd.dma_gather(
                        out_ap=hvb[:].rearrange("p (c d) -> p c d", d=D),
                        in_ap=dtab[g], idxs_ap=db_sb[:, isl],
                        num_idxs=NI, num_idxs_reg=NI, elem_size=D,
                    )
                    hu3 = hu[:].rearrange("p (c d) -> p c d", d=2 * D)
                    nc.vector.tensor_mul(out=hva[:], in0=hva[:],
                                         in1=hu3[:, :, :D])
                    nc.vector.tensor_mul(out=hvb[:], in0=hvb[:],
                                         in1=hu3[:, :, D:])
                    nc.vector.tensor_reduce(
                        out=out_sb[:, (2 * t2) * C:(2 * t2 + 1) * C],
                        in_=hva[:].rearrange("p (c d) -> p c d", d=D),
                        axis=mybir.AxisListType.X, op=mybir.AluOpType.add)
                    nc.vector.tensor_reduce(
                        out=out_sb[:, (2 * t2 + 1) * C:(2 * t2 + 2) * C],
                        in_=hvb[:].rearrange("p (c d) -> p c d", d=D),
                        axis=mybir.AxisListType.X, op=mybir.AluOpType.add)
                    t2 += 1

            # single tiles
            t1 = 0
            for g in range(4):
                for _ in range(t1g[g]):
                    isl = slice(t1 * S, (t1 + 1) * S)
                    hu = gpool.tile([P, NI], mybir.dt.float32, tag="hu1")
                    hv = gpool.tile([P, NI], mybir.dt.float32, tag="hv1")
                    nc.gpsimd.dma_gather(
                        out_ap=hu[:].rearrange("p (c d) -> p c d", d=D),
                        in_ap=tab1[g], idxs_ap=ss_sb[:, isl],
                        num_idxs=NI, num_idxs_reg=NI, elem_size=D,
                    )
                    nc.gpsimd.dma_gather(
                        out_ap=hv[:].rearrange("p (c d) -> p c d", d=D),
                        in_ap=dtab[g], idxs_ap=ds_sb[:, isl],
                        num_idxs=NI, num_idxs_reg=NI, elem_size=D,
                    )
                    nc.vector.tensor_mul(out=hu[:], in0=hu[:], in1=hv[:])
                    nc.vector.tensor_reduce(
                        out=out_sb[:, (2 * T2 + t1) * C:(2 * T2 + t1 + 1) * C],
                        in_=hu[:].rearrange("p (c d) -> p c d", d=D),
                        axis=mybir.AxisListType.X, op=mybir.AluOpType.add)
                    t1 += 1
            nc.sync.dma_start(out=out, in_=out_sb[:])
    nc.compile()
    return nc


def _get_nc(cfg):
    key = (tuple(cfg[0]), tuple(cfg[1]))
    nc = _CACHE.get(key)
    if nc is None:
        nc = _build(key)
        _CACHE[key] = nc
    return nc


def _pair_decompose(s, d, eids):
    """Greedy consecutive-row pairing of one bucket's edges.

    Returns (pa, pb, singles): edge-id arrays; s[pb] == s[pa] + 1."""
    o = np.argsort(s, kind="stable")
    ss = s[o]
    rows, starts, cnts = np.unique(ss, return_index=True, return_counts=True)
    pa, pb, singles = [], [], []
    carry = np.empty(0, dtype=np.int64)
    prev = -2
    for r, st, c in zip(rows.tolist(), starts.tolist(), cnts.tolist()):
        cur = o[st:st + c]
        if r == prev + 1 and len(carry):
            m = min(len(carry), c)
            pa.append(carry[:m])
            pb.append(cur[:m])
            if len(carry) > m:
                singles.append(carry[m:])
            carry = cur[m:]
        else:
            if len(carry):
                singles.append(carry)
            carry = cur
        prev = r
    if len(carry):
        singles.append(carry)
    cat = lambda lst: (np.concatenate(lst) if lst else np.empty(0, dtype=np.int64))
    pa, pb, singles = cat(pa), cat(pb), cat(singles)
    return eids[pa], eids[pb], eids[singles]


def _wrap_idx(vals, ntiles):
    """[ntiles*NI] int array -> [128, ntiles*S] int16 wrapped layout."""
    v16 = vals.astype(np.uint16).view(np.int16).reshape(ntiles, S, 16)
    blk = v16.transpose(2, 0, 1).reshape(16, ntiles * S)
    return np.tile(blk, (8, 1))


def _prepare_core(s, d):
    """Bucket + pair-decompose one core's edges.

    Returns dict with idx arrays (unpadded, per group) and bookkeeping."""
    grp = (s >= OFF).astype(np.int8) * 2 + (d >= OFF).astype(np.int8)
    per_group = []
    for g in range(4):
        eids = np.where(grp == g)[0]
        sg = s[eids] - OFF * (g >> 1)
        pa, pb, single = _pair_decompose(sg, None, eids)
        per_group.append({
            "pa": pa, "pb": pb, "single": single,
            "soff": OFF * (g >> 1), "doff": OFF * (g & 1),
        })
    return per_group


def _core_arrays(s, d, per_group, t2g, t1g):
    """Build padded idx arrays for one core given global tile counts."""
    T2, T1 = sum(t2g), sum(t1g)
    sp = np.zeros(max(T2, 1) * NI, dtype=np.int32)
    da = np.zeros(max(T2, 1) * NI, dtype=np.int32)
    db = np.zeros(max(T2, 1) * NI, dtype=np.int32)
    ss_ = np.zeros(max(T1, 1) * NI, dtype=np.int32)
    ds_ = np.zeros(max(T1, 1) * NI, dtype=np.int32)
    b2 = b1 = 0
    for g in range(4):
        pg = per_group[g]
        n2, n1 = len(pg["pa"]), len(pg["single"])
        sp[b2:b2 + n2] = s[pg["pa"]] - pg["soff"]
        da[b2:b2 + n2] = d[pg["pa"]] - pg["doff"]
        db[b2:b2 + n2] = d[pg["pb"]] - pg["doff"]
        ss_[b1:b1 + n1] = s[pg["single"]] - pg["soff"]
        ds_[b1:b1 + n1] = d[pg["single"]] - pg["doff"]
        b2 += t2g[g] * NI
        b1 += t1g[g] * NI
    return (
        _wrap_idx(sp, max(T2, 1)), _wrap_idx(da, max(T2, 1)),
        _wrap_idx(db, max(T2, 1)), _wrap_idx(ss_, max(T1, 1)),
        _wrap_idx(ds_, max(T1, 1)),
    )


def _unpermute_core(out, per_group, t2g, t1g):
    T2, T1 = sum(t2g), sum(t1g)
    ncols = (2 * T2 + T1) * C
    # slot j of pair tile t2 -> scores at out[j%128, (2*t2 + {0,1})*C + (j%NI)//128]
    res = np.empty(E_CORE, dtype=np.float32)
    o3 = out.reshape(P, ncols // C, C)
    b2 = b1 = 0
    for g in range(4):
        pg = per_group[g]
        n2, n1 = len(pg["pa"]), len(pg["single"])
        j = b2 + np.arange(n2)
        t_arr = j // NI
        res[pg["pa"]] = o3[j % P, 2 * t_arr, (j % NI) // P]
        res[pg["pb"]] = o3[j % P, 2 * t_arr + 1, (j % NI) // P]
        j1 = b1 + np.arange(n1)
        res[pg["single"]] = o3[j1 % P, 2 * T2 + j1 // NI, (j1 % NI) // P]
        b2 += t2g[g] * NI
        b1 += t1g[g] * NI
    return res


def kernel(h, src_idx, dst_idx):
    h = np.ascontiguousarray(np.asarray(h, dtype=np.float32))
    src = np.asarray(src_idx).astype(np.int32).reshape(N_CORES, E_CORE)
    dst = np.asarray(dst_idx).astype(np.int32).reshape(N_CORES, E_CORE)

    pgs = [_prepare_core(src[c], dst[c]) for c in range(N_CORES)]
    t2g = [0, 0, 0, 0]
    t1g = [0, 0, 0, 0]
    for c in range(N_CORES):
        for g in range(4):
            t2g[g] = max(t2g[g], -(-len(pgs[c][g]["pa"]) // NI))
            t1g[g] = max(t1g[g], -(-len(pgs[c][g]["single"]) // NI))
    cfg = (tuple(t2g), tuple(t1g))

    nc = _get_nc(cfg)
    in_maps = []
    for c in range(N_CORES):
        sp, da, db, ss_, ds_ = _core_arrays(src[c], dst[c], pgs[c], t2g, t1g)
        in_maps.append({"h": h, "sp": sp, "da": da, "db": db, "ss": ss_, "ds": ds_})
    res = run_bass_kernel_spmd(nc, in_maps, core_ids=list(range(N_CORES)))
    outs = [
        _unpermute_core(np.asarray(res.results[c]["out"], dtype=np.float32),
                        pgs[c], t2g, t1g)
        for c in range(N_CORES)
    ]
    return np.concatenate(outs).reshape(N_EDGES, 1)



# revision 3
# speedup vs baseline: 4.4523x; 4.4523x over previous
"""Trainium2 Bass kernel: per-edge dot product (u_dot_v GNN predictor).

score[e] = sum_d h[src[e], d] * h[dst[e], d]   -> [E, 1] float32

Strategy (edge-parallel over 8 cores):
  - Each core gets E/8 = 80000 edges; the full node table h stays in HBM and
    rows are fetched per edge with the GPSIMD dma_gather instruction.
    The kernel is bound by the Q7 descriptor-generation rate (~8 ns/descriptor),
    so the design minimizes descriptor count.
  - dma_gather indices are int16, so node ids >= 32768 can't be addressed
    directly. Host buckets each core's edges 4 ways by (src >= 32768,
    dst >= 32768); each bucket's gathers use base-offset views of h with
    rebased indices.
  - Descriptor reduction: within each bucket, edges whose src rows are
    CONSECUTIVE (r, r+1) are paired; one elem_size=256/elem_step=128
    descriptor fetches both rows (overlapping-window source AP), so a pair
    of edges costs 1 src descriptor instead of 2. ~70% of edges pair up.
  - Per tile of 1024 descriptors: gather h[src] (paired or single) and the
    matching h[dst] rows, DVE multiply + segmented reduce over the feature
    axis -> [128, 8] scores per 1024 edge-slots.
  - Host un-permutes the bucketed scores back to edge order.
"""

import numpy as np

import concourse.bacc as bacc
import concourse.mybir as mybir
import concourse.tile as tile
from concourse import bass
from concourse.bass_utils import run_bass_kernel_spmd

N_NODES = 50000
D = 128
N_EDGES = 640000
N_CORES = 8
P = 128
E_CORE = N_EDGES // N_CORES  # 80000
NI = 1024                    # descriptors per dma_gather tile
C = NI // P                  # 8 chunks per partition
S = NI // 16                 # idx columns per 16-partition block
OFF = 32768                  # int16 index range boundary

_CACHE: dict = {}


def _h_tables(nc, h):
    """Per-bucket source APs: (single-row table, overlapping pair table)."""
    lo_rows, hi_rows = OFF, N_NODES - OFF
    h_lo1 = h[:OFF, :]
    h_hi1 = h[OFF:, :]
    # overlapping-window pair tables: pseudo-row r = elements 128r..128r+255
    # (rows r and r+1). Base stride 128 elems, window 256 elems.
    h_lo2 = bass.AP(h.tensor, 0, [[D, lo_rows - 1], [1, 2 * D]])
    h_hi2 = bass.AP(h.tensor, OFF * D, [[D, hi_rows - 1], [1, 2 * D]])
    return (h_lo1, h_hi1, h_lo2, h_hi2)


def _build(cfg):
    """cfg = (t2 per group [4], t1 per group [4]) tile counts."""
    t2g, t1g = cfg
    T2, T1 = sum(t2g), sum(t1g)
    nc = bacc.Bacc(
        "TRN2",
        target_bir_lowering=False,
        debug=False,
        enable_asserts=False,
        num_devices=N_CORES,
        num_swdge_queues=4,
    )
    qctr = [0]

    def next_q():
        q = qctr[0] % 4
        qctr[0] += 1
        return q
    h = nc.dram_tensor("h", [N_NODES, D], mybir.dt.float32, kind="ExternalInput").ap()

    def idx_in(name, ntiles):
        return nc.dram_tensor(name, [P, max(ntiles, 1) * S], mybir.dt.int16,
                              kind="ExternalInput").ap()

    sp = idx_in("sp", T2)    # pair src bases
    da = idx_in("da", T2)    # pair dst idx, first edge of pair
    db = idx_in("db", T2)    # pair dst idx, second edge
    ss = idx_in("ss", T1)    # single src idx
    ds = idx_in("ds", T1)    # single dst idx
    ncols = (2 * T2 + T1) * C
    out = nc.dram_tensor("out", [P, ncols], mybir.dt.float32, kind="ExternalOutput").ap()

    h_lo1, h_hi1, h_lo2, h_hi2 = _h_tables(nc, h)
    tab1 = [h_lo1, h_lo1, h_hi1, h_hi1]   # src single table per group
    tab2 = [h_lo2, h_lo2, h_hi2, h_hi2]   # src pair table per group
    dtab = [h_lo1, h_hi1, h_lo1, h_hi1]   # dst table per group

    with tile.TileContext(nc) as tc:
        with (
            tc.tile_pool(name="idx", bufs=1) as ipool,
            tc.tile_pool(name="gath", bufs=4) as gpool,
            tc.tile_pool(name="res", bufs=1) as rpool,
        ):
            def load_idx(ap_dram, ntiles, tag):
                t = ipool.tile([P, max(ntiles, 1) * S], mybir.dt.int16, tag=tag)
                nc.sync.dma_start(out=t[:], in_=ap_dram)
                return t

            sp_sb = load_idx(sp, T2, "sp")
            da_sb = load_idx(da, T2, "da")
            db_sb = load_idx(db, T2, "db")
            ss_sb = load_idx(ss, T1, "ss")
            ds_sb = load_idx(ds, T1, "ds")
            out_sb = rpool.tile([P, ncols], mybir.dt.float32)

            # pair tiles
            t2 = 0
            for g in range(4):
                for _ in range(t2g[g]):
                    isl = slice(t2 * S, (t2 + 1) * S)
                    hu = gpool.tile([P, 2 * NI], mybir.dt.float32, tag="hu2")
                    hva = gpool.tile([P, NI], mybir.dt.float32, tag="hva")
                    hvb = gpool.tile([P, NI], mybir.dt.float32, tag="hvb")
                    nc.gpsimd.dma_gather(
                        out_ap=hu[:].rearrange("p (c d) -> p c d", d=2 * D),
                        in_ap=tab2[g], idxs_ap=sp_sb[:, isl],
                        num_idxs=NI, num_idxs_reg=NI,
                        elem_size=2 * D, elem_step=D, queue_num=next_q(),
                    )
                    nc.gpsimd.dma_gather(
                        out_ap=hva[:].rearrange("p (c d) -> p c d", d=D),
                        in_ap=dtab[g], idxs_ap=da_sb[:, isl],
                        num_idxs=NI, num_idxs_reg=NI, elem_size=D,
                        queue_num=next_q(),
                    )
                    nc.gpsimd.dma_gather(
                        out_ap=hvb[:].rearrange("p (c d) -> p c d", d=D),
                        in_ap=dtab[g], idxs_ap=db_sb[:, isl],
                        num_idxs=NI, num_idxs_reg=NI, elem_size=D,
                        queue_num=next_q(),
                    )
                    hu3 = hu[:].rearrange("p (c d) -> p c d", d=2 * D)
                    nc.vector.tensor_mul(out=hva[:], in0=hva[:],
                                         in1=hu3[:, :, :D])
                    nc.vector.tensor_mul(out=hvb[:], in0=hvb[:],
                                         in1=hu3[:, :, D:])
                    nc.vector.tensor_reduce(
                        out=out_sb[:, (2 * t2) * C:(2 * t2 + 1) * C],
                        in_=hva[:].rearrange("p (c d) -> p c d", d=D),
                        axis=mybir.AxisListType.X, op=mybir.AluOpType.add)
                    nc.vector.tensor_reduce(
                        out=out_sb[:, (2 * t2 + 1) * C:(2 * t2 + 2) * C],
                        in_=hvb[:].rearrange("p (c d) -> p c d", d=D),
                        axis=mybir.AxisListType.X, op=mybir.AluOpType.add)
                    t2 += 1

            # single tiles
            t1 = 0
            for g in range(4):
                for _ in range(t1g[g]):
                    isl = slice(t1 * S, (t1 + 1) * S)
                    hu = gpool.tile([P, NI], mybir.dt.float32, tag="hu1")
                    hv = gpool.tile([P, NI], mybir.dt.float32, tag="hv1")
                    nc.gpsimd.dma_gather(
                        out_ap=hu[:].rearrange("p (c d) -> p c d", d=D),
                        in_ap=tab1[g], idxs_ap=ss_sb[:, isl],
                        num_idxs=NI, num_idxs_reg=NI, elem_size=D,
                        queue_num=next_q(),
                    )
                    nc.gpsimd.dma_gather(
                        out_ap=hv[:].rearrange("p (c d) -> p c d", d=D),
                        in_ap=dtab[g], idxs_ap=ds_sb[:, isl],
                        num_idxs=NI, num_idxs_reg=NI, elem_size=D,
                        queue_num=next_q(),
                    )
                    nc.vector.tensor_mul(out=hu[:], in0=hu[:], in1=hv[:])
                    nc.vector.tensor_reduce(
                        out=out_sb[:, (2 * T2 + t1) * C:(2 * T2 + t1 + 1) * C],
                        in_=hu[:].rearrange("p (c d) -> p c d", d=D),
                        axis=mybir.AxisListType.X, op=mybir.AluOpType.add)
                    t1 += 1
            nc.sync.dma_start(out=out, in_=out_sb[:])
    nc.compile()
    return nc


def _get_nc(cfg):
    key = (tuple(cfg[0]), tuple(cfg[1]))
    nc = _CACHE.get(key)
    if nc is None:
        nc = _build(key)
        _CACHE[key] = nc
    return nc


def _pair_decompose(s, d, eids):
    """Greedy consecutive-row pairing of one bucket's edges.

    Returns (pa, pb, singles): edge-id arrays; s[pb] == s[pa] + 1."""
    o = np.argsort(s, kind="stable")
    ss = s[o]
    rows, starts, cnts = np.unique(ss, return_index=True, return_counts=True)
    pa, pb, singles = [], [], []
    carry = np.empty(0, dtype=np.int64)
    prev = -2
    for r, st, c in zip(rows.tolist(), starts.tolist(), cnts.tolist()):
        cur = o[st:st + c]
        if r == prev + 1 and len(carry):
            m = min(len(carry), c)
            pa.append(carry[:m])
            pb.append(cur[:m])
            if len(carry) > m:
                singles.append(carry[m:])
            carry = cur[m:]
        else:
            if len(carry):
                singles.append(carry)
            carry = cur
        prev = r
    if len(carry):
        singles.append(carry)
    cat = lambda lst: (np.concatenate(lst) if lst else np.empty(0, dtype=np.int64))
    pa, pb, singles = cat(pa), cat(pb), cat(singles)
    return eids[pa], eids[pb], eids[singles]


def _wrap_idx(vals, ntiles):
    """[ntiles*NI] int array -> [128, ntiles*S] int16 wrapped layout."""
    v16 = vals.astype(np.uint16).view(np.int16).reshape(ntiles, S, 16)
    blk = v16.transpose(2, 0, 1).reshape(16, ntiles * S)
    return np.tile(blk, (8, 1))


def _prepare_core(s, d):
    """Bucket + pair-decompose one core's edges.

    Returns dict with idx arrays (unpadded, per group) and bookkeeping."""
    grp = (s >= OFF).astype(np.int8) * 2 + (d >= OFF).astype(np.int8)
    per_group = []
    for g in range(4):
        eids = np.where(grp == g)[0]
        sg = s[eids] - OFF * (g >> 1)
        pa, pb, single = _pair_decompose(sg, None, eids)
        per_group.append({
            "pa": pa, "pb": pb, "single": single,
            "soff": OFF * (g >> 1), "doff": OFF * (g & 1),
        })
    return per_group


def _core_arrays(s, d, per_group, t2g, t1g):
    """Build padded idx arrays for one core given global tile counts."""
    T2, T1 = sum(t2g), sum(t1g)
    sp = np.zeros(max(T2, 1) * NI, dtype=np.int32)
    da = np.zeros(max(T2, 1) * NI, dtype=np.int32)
    db = np.zeros(max(T2, 1) * NI, dtype=np.int32)
    ss_ = np.zeros(max(T1, 1) * NI, dtype=np.int32)
    ds_ = np.zeros(max(T1, 1) * NI, dtype=np.int32)
    b2 = b1 = 0
    for g in range(4):
        pg = per_group[g]
        n2, n1 = len(pg["pa"]), len(pg["single"])
        sp[b2:b2 + n2] = s[pg["pa"]] - pg["soff"]
        da[b2:b2 + n2] = d[pg["pa"]] - pg["doff"]
        db[b2:b2 + n2] = d[pg["pb"]] - pg["doff"]
        ss_[b1:b1 + n1] = s[pg["single"]] - pg["soff"]
        ds_[b1:b1 + n1] = d[pg["single"]] - pg["doff"]
        b2 += t2g[g] * NI
        b1 += t1g[g] * NI
    return (
        _wrap_idx(sp, max(T2, 1)), _wrap_idx(da, max(T2, 1)),
        _wrap_idx(db, max(T2, 1)), _wrap_idx(ss_, max(T1, 1)),
        _wrap_idx(ds_, max(T1, 1)),
    )


def _unpermute_core(out, per_group, t2g, t1g):
    T2, T1 = sum(t2g), sum(t1g)
    ncols = (2 * T2 + T1) * C
    # slot j of pair tile t2 -> scores at out[j%128, (2*t2 + {0,1})*C + (j%NI)//128]
    res = np.empty(E_CORE, dtype=np.float32)
    o3 = out.reshape(P, ncols // C, C)
    b2 = b1 = 0
    for g in range(4):
        pg = per_group[g]
        n2, n1 = len(pg["pa"]), len(pg["single"])
        j = b2 + np.arange(n2)
        t_arr = j // NI
        res[pg["pa"]] = o3[j % P, 2 * t_arr, (j % NI) // P]
        res[pg["pb"]] = o3[j % P, 2 * t_arr + 1, (j % NI) // P]
        j1 = b1 + np.arange(n1)
        res[pg["single"]] = o3[j1 % P, 2 * T2 + j1 // NI, (j1 % NI) // P]
        b2 += t2g[g] * NI
        b1 += t1g[g] * NI
    return res


def kernel(h, src_idx, dst_idx):
    h = np.ascontiguousarray(np.asarray(h, dtype=np.float32))
    src = np.asarray(src_idx).astype(np.int32).reshape(N_CORES, E_CORE)
    dst = np.asarray(dst_idx).astype(np.int32).reshape(N_CORES, E_CORE)

    pgs = [_prepare_core(src[c], dst[c]) for c in range(N_CORES)]
    t2g = [0, 0, 0, 0]
    t1g = [0, 0, 0, 0]
    for c in range(N_CORES):
        for g in range(4):
            t2g[g] = max(t2g[g], -(-len(pgs[c][g]["pa"]) // NI))
            t1g[g] = max(t1g[g], -(-len(pgs[c][g]["single"]) // NI))
    cfg = (tuple(t2g), tuple(t1g))

    nc = _get_nc(cfg)
    in_maps = []
    for c in range(N_CORES):
        sp, da, db, ss_, ds_ = _core_arrays(src[c], dst[c], pgs[c], t2g, t1g)
        in_maps.append({"h": h, "sp": sp, "da": da, "db": db, "ss": ss_, "ds": ds_})
    res = run_bass_kernel_spmd(nc, in_maps, core_ids=list(range(N_CORES)))
    outs = [
        _unpermute_core(np.asarray(res.results[c]["out"], dtype=np.float32),
                        pgs[c], t2g, t1g)
        for c in range(N_CORES)
    ]
    return np.concatenate(outs).reshape(N_EDGES, 1)



# revision 4
# speedup vs baseline: 4.4765x; 1.0054x over previous
"""Trainium2 Bass kernel: per-edge dot product (u_dot_v GNN predictor).

score[e] = sum_d h[src[e], d] * h[dst[e], d]   -> [E, 1] float32

Strategy (src-range sharding + PE expansion for src, per-edge gather for dst):
  - Edges are assigned to cores by src range: core c owns src rows
    [c*6272, (c+1)*6272). The core's src slab (49 blocks x 128 rows) is
    DMA'd sequentially into SBUF once and converted to bf16 - the src side
    costs NO gather descriptors and NO random HBM reads.
  - Edges are processed in tiles of 128 slots, each tile confined to one
    128-row src block b. A host-built one-hot matrix E [128 rows x 128
    slots] (bf16) expands block rows to per-slot src features with one PE
    matmul per tile: hu_exp = E.T @ slab[b] -> PSUM [128 slots x 128 feat].
  - The dst side is a per-edge dma_gather (f32 rows from HBM), 2-way
    bucketed by dst >= 32768 (int16 gather indices), NI=1024 slots per
    instruction, spread across 4 SWDGE queues so Q7 descriptor generation
    runs on all four core pairs concurrently.
  - DVE: prod = hv * hu_exp (SBUF x PSUM) -> bf16, segmented reduce over
    the feature axis -> per-slot scores [128, n_tiles]. Host un-permutes.

Slot layout (per core): slots are grouped [bucket 0 | bucket 1]; within a
bucket, by src block. Segment sizes (in tiles) per (bucket, block) are the
max over cores, so the block schedule is compile-time uniform (SPMD).
Slot j lands at SBUF partition j%128, tile j//128 (gather wrap layout).
"""

import numpy as np
import ml_dtypes

import concourse.bacc as bacc
import concourse.mybir as mybir
import concourse.tile as tile
from concourse.bass_utils import run_bass_kernel_spmd

N_NODES = 50000
D = 128
N_EDGES = 640000
N_CORES = 8
P = 128
BLK = 128                    # src block rows (PE contract dim)
BPC = 49                     # blocks per core
SPAN = BLK * BPC             # 6272 src rows per core
NI = 1024                    # slots per compute subgroup
C = NI // P                  # 8 tiles per compute subgroup
S = NI // 16                 # idx columns per 16-partition block
GNI = 1024                   # slots per dma_gather instruction
GC = GNI // P                # tiles per gather instruction
GS = GNI // 16               # idx columns per gather instruction
OFF = 32768                  # int16 index range boundary

_CACHE: dict = {}


def _wrap_idx(vals, ngath):
    """[ngath*GNI] int array -> [128, ngath*GS] int16 wrapped layout."""
    v16 = vals.astype(np.uint16).view(np.int16).reshape(ngath, GS, 16)
    blk = v16.transpose(2, 0, 1).reshape(16, ngath * GS)
    return np.tile(blk, (8, 1))


def _schedule(tiles_b):
    """tiles_b: [2, BPC] per-(bucket, block) tile counts.

    Returns (blk_of_tile list, groups_b0, n_tiles); bucket segments are
    padded to whole gather groups (C tiles).
    """
    blk_of_tile = []
    groups = []
    for bkt in range(2):
        seg = []
        for b in range(BPC):
            seg += [b] * tiles_b[bkt][b]
        pad = (-len(seg)) % GC
        seg += [0] * pad
        groups.append(len(seg) // GC)
        blk_of_tile += seg
    return blk_of_tile, groups[0], len(blk_of_tile)


def _prepare_core(s, d, c, tiles_b, groups_b0, n_tiles):
    """Slot assignment for one core given the uniform schedule.

    Returns dict with eid (int64 [n_slots], -1 padding), u (row within
    block), dst (rebased dst idx), n_slots.
    """
    base = c * SPAN
    blk_id = (s - base) // BLK
    u = (s - base) % BLK
    bucket = (d >= OFF).astype(np.int8)

    n_slots = n_tiles * P
    eid = np.full(n_slots, -1, np.int64)
    uu = np.zeros(n_slots, np.int32)
    dd = np.zeros(n_slots, np.int32)
    pos = 0
    for bkt in range(2):
        for b in range(BPC):
            sel = np.flatnonzero((bucket == bkt) & (blk_id == b))
            n = len(sel)
            eid[pos:pos + n] = sel
            uu[pos:pos + n] = u[sel]
            dd[pos:pos + n] = d[sel] - bkt * OFF
            pos += tiles_b[bkt][b] * P
        pos = groups_b0 * GNI if bkt == 0 else n_tiles * P
    return {"eid": eid, "u": uu, "dst": dd, "n_slots": n_slots}


def _core_arrays(prep, n_tiles):
    """Device input arrays for one core: (didx int16 wrapped, E bf16)."""
    ngath = n_tiles * P // GNI
    didx = _wrap_idx(prep["dst"], ngath)
    E = np.zeros((P, n_tiles * P), np.float32)
    slot = np.arange(n_tiles * P)
    E[prep["u"], slot] = 1.0
    # zero out padding columns so pad slots produce 0 (not h[blk][0])
    E[0, prep["eid"] < 0] = 0.0
    return didx, E.astype(ml_dtypes.float8_e4m3)


def _unpermute_core(out, prep):
    """out: [128, n_tiles] f32 device scores -> scores per core-edge."""
    j = np.arange(prep["n_slots"])
    scores = out[j % P, j // P]
    valid = prep["eid"] >= 0
    res = np.empty(np.count_nonzero(valid), np.float32)
    res[prep["eid"][valid]] = scores[valid]
    return res


def _build(cfg):
    """cfg = (tiles_b0 tuple, tiles_b1 tuple)."""
    tiles_b = np.array(cfg)
    blk_of_tile, groups_b0, n_tiles = _schedule(tiles_b)
    ngroups = n_tiles // C
    ngath = n_tiles // GC
    nc = bacc.Bacc(
        "TRN2",
        target_bir_lowering=False,
        debug=False,
        enable_asserts=False,
        num_devices=N_CORES,
        num_swdge_queues=4,
    )
    h = nc.dram_tensor("h", [N_NODES, D], mybir.dt.float32,
                       kind="ExternalInput").ap()
    hslab = nc.dram_tensor("hslab", [SPAN, D], mybir.dt.float32,
                           kind="ExternalInput").ap()
    didx = nc.dram_tensor("didx", [P, ngath * GS], mybir.dt.int16,
                          kind="ExternalInput").ap()
    Emat = nc.dram_tensor("E", [P, n_tiles * P], mybir.dt.float8e4,
                          kind="ExternalInput").ap()
    out = nc.dram_tensor("out", [P, n_tiles], mybir.dt.float32,
                         kind="ExternalOutput").ap()

    h_lo = h[:OFF, :]
    h_hi = h[OFF:, :]

    with tile.TileContext(nc) as tc:
        with (
            tc.tile_pool(name="resid", bufs=1) as rpool,
            tc.tile_pool(name="emat", bufs=4) as epool,
            tc.tile_pool(name="gath", bufs=6) as gpool,
            tc.tile_pool(name="psum", bufs=3, space="PSUM") as ppool,
            tc.tile_pool(name="prod", bufs=4) as prpool,
        ):
            slab_f32 = rpool.tile([P, BPC * D], mybir.dt.float32)
            nc.sync.dma_start(
                out=slab_f32[:].rearrange("p (b f) -> p b f", f=D),
                in_=hslab.rearrange("(b p) f -> p b f", p=P),
            )
            slab = rpool.tile([P, BPC * D], mybir.dt.bfloat16)
            nc.vector.tensor_copy(out=slab[:], in_=slab_f32[:])
            slab3 = slab[:].rearrange("p (b f) -> p b f", f=D)

            didx_sb = rpool.tile([P, ngath * GS], mybir.dt.int16)
            nc.sync.dma_start(out=didx_sb[:], in_=didx)
            out_sb = rpool.tile([P, n_tiles], mybir.dt.float32)

            for gg in range(ngath):
                esb = epool.tile([P, GNI], mybir.dt.float8e4, tag="E")
                nc.sync.dma_start(out=esb[:],
                                  in_=Emat[:, gg * GNI:(gg + 1) * GNI])
                hv = gpool.tile([P, GNI], mybir.dt.float32, tag="hv")
                nc.gpsimd.dma_gather(
                    out_ap=hv[:].rearrange("p (c d) -> p c d", d=D),
                    in_ap=(h_lo if gg < groups_b0 else h_hi),
                    idxs_ap=didx_sb[:, gg * GS:(gg + 1) * GS],
                    num_idxs=GNI, num_idxs_reg=GNI, elem_size=D,
                    queue_num=gg % 4,
                )
                for half in range(GNI // NI):
                    g = gg * (GNI // NI) + half
                    hsl = slice(half * NI, (half + 1) * NI)
                    hu = ppool.tile([P, C, P], mybir.dt.float32, space="PSUM")
                    for k in range(C):
                        nc.tensor.matmul(
                            out=hu[:, k, :],
                            lhsT=esb[:, half * NI + k * P:
                                     half * NI + (k + 1) * P],
                            rhs=slab3[:, blk_of_tile[g * C + k], :],
                            start=True, stop=True,
                        )
                    prod = prpool.tile([P, NI], mybir.dt.bfloat16, tag="prod")
                    nc.vector.tensor_mul(out=prod[:], in0=hv[:, hsl],
                                         in1=hu[:].rearrange("p c d -> p (c d)"))
                    nc.vector.tensor_reduce(
                        out=out_sb[:, g * C:(g + 1) * C],
                        in_=prod[:].rearrange("p (c d) -> p c d", d=D),
                        axis=mybir.AxisListType.X, op=mybir.AluOpType.add)

            nc.sync.dma_start(out=out, in_=out_sb[:])
    nc.compile()
    return nc


def _get_nc(cfg):
    nc = _CACHE.get(cfg)
    if nc is None:
        nc = _build(cfg)
        _CACHE[cfg] = nc
    return nc


def _host_prep(src, dst):
    """Shared host-side prep: per-core slot schedules and input arrays."""
    core_of = src // SPAN
    orders, counts = [], np.zeros((N_CORES, 2, BPC), np.int64)
    for c in range(N_CORES):
        sel = np.flatnonzero(core_of == c)
        orders.append(sel)
        s, d = src[sel], dst[sel]
        blk_id = (s - c * SPAN) // BLK
        bucket = (d >= OFF).astype(np.int64)
        np.add.at(counts[c], (bucket, blk_id), 1)
    tiles_b = -(-counts.max(axis=0) // P)  # [2, BPC]
    _, groups_b0, n_tiles = _schedule(tiles_b)
    preps = [
        _prepare_core(src[orders[c]], dst[orders[c]], c, tiles_b,
                      groups_b0, n_tiles)
        for c in range(N_CORES)
    ]
    cfg = (tuple(int(x) for x in tiles_b[0]), tuple(int(x) for x in tiles_b[1]))
    return orders, preps, cfg, n_tiles


def _assemble(h, preps, n_tiles):
    in_maps = []
    hpad = np.zeros((N_CORES * SPAN, D), np.float32)
    hpad[:N_NODES] = h
    for c in range(N_CORES):
        didx, E = _core_arrays(preps[c], n_tiles)
        in_maps.append({
            "h": h, "hslab": hpad[c * SPAN:(c + 1) * SPAN], "didx": didx,
            "E": E,
        })
    return in_maps


def kernel(h, src_idx, dst_idx):
    h = np.ascontiguousarray(np.asarray(h, dtype=np.float32))
    src = np.asarray(src_idx).astype(np.int32)
    dst = np.asarray(dst_idx).astype(np.int32)

    orders, preps, cfg, n_tiles = _host_prep(src, dst)
    nc = _get_nc(cfg)
    in_maps = _assemble(h, preps, n_tiles)
    res = run_bass_kernel_spmd(nc, in_maps, core_ids=list(range(N_CORES)))

    scores = np.empty(N_EDGES, np.float32)
    for c in range(N_CORES):
        out = np.asarray(res.results[c]["out"], dtype=np.float32)
        scores[orders[c]] = _unpermute_core(out, preps[c])
    return scores.reshape(N_EDGES, 1)
